# revision 36
# baseline (speedup 1.0000x reference)
"""Trainium2 Bass kernel for nn_Actor_56916906607124 (compute_encoder_mask).

Computation (per batch instance b, row i):
  mask[b,i,j] = 1 iff  (j is among the 16 nearest time-window-compatible,
                        non-diagonal neighbors of i)  OR depot[b,i]  OR
                        depot[b,j]  OR i == j.

Sharding: pure data parallelism — batch B=8 across 8 NeuronCores, one
instance per core.  No collectives.

Per-core algorithm (N=2048, 16 row-tiles of 128 rows):
  selection key  x = twc - d  (eligible j: x in (0,1]; blocked j: x <= 0);
  diagonal forced out of selection (x -= 2.5 on the diag block).
  Selection of the 16th-largest x per row: 8 chunk-wise DVE `max` (top-8 of
  each 256-col chunk) -> 64 candidates, global `max`, `match_replace` removes
  those 8, `max` again -> v8b[:,7] = 16th largest.  A chunk holding >8 of the
  true top-16 (rare) under-estimates the threshold, which the count check
  below catches.
  knn = (x >= t16 - 1e30*depot_row)  [depot rows -> all 1], with a per-row
  count (accum_out) shipped out; out = knn OR depot_col OR diag, built in
  bf16 (exact for 0/1, 2x DVE mode), widened to f32 on the Scalar engine.
  Host repairs rows whose count != 16 (ignoring depot rows) by exact numpy
  recomputation -- float ties at the 8/9 or 16/17 selection boundary and
  chunk mis-coverage (O(N) per flagged row; ~1-3 rows on the seed-0 data).
"""

from contextlib import ExitStack

import numpy as np

import concourse.bass as bass
import concourse.mybir as mybir
from concourse import bacc, tile
from concourse.bass_utils import run_bass_kernel_spmd

B, N, P = 8, 2048, 128
NT = N // P  # 16 row-tiles per core
K = 16
f32 = mybir.dt.float32
i32 = mybir.dt.int32
bf16 = mybir.dt.bfloat16
Alu = mybir.AluOpType
Act = mybir.ActivationFunctionType

_program_cache = {}


def build_program():
    if "nc" in _program_cache:
        return _program_cache["nc"]
    nc = bacc.Bacc()
    d_h = nc.declare_dram_parameter("d", [N, N], f32, isOutput=False)
    twc_h = nc.declare_dram_parameter("twc", [N, N], i32, isOutput=False)
    dflat_h = nc.declare_dram_parameter("dflat", [1, N], bf16, isOutput=False)
    dbig_h = nc.declare_dram_parameter("dbig", [P, NT], f32, isOutput=False)
    ident_h = nc.declare_dram_parameter("ident", [P, P], bf16, isOutput=False)
    mask_h = nc.declare_dram_parameter("mask", [N, N], f32, isOutput=True)
    nge_h = nc.declare_dram_parameter("nge", [P, NT], f32, isOutput=True)
    t16r_h = nc.declare_dram_parameter("t16r", [P, NT], f32, isOutput=True)

    with ExitStack() as ctx:
        tc = ctx.enter_context(tile.TileContext(nc))
        const = ctx.enter_context(tc.tile_pool(name="const", bufs=1))
        inp = ctx.enter_context(tc.tile_pool(name="inp", bufs=5))
        work = ctx.enter_context(tc.tile_pool(name="work", bufs=3))
        outp = ctx.enter_context(tc.tile_pool(name="outp", bufs=3))
        small = ctx.enter_context(tc.tile_pool(name="small", bufs=4))
        psum = ctx.enter_context(
            tc.tile_pool(name="psum", bufs=2, space="PSUM"))

        # build dc01 (depot broadcast across partitions) on-chip: K=1 matmul
        # ones[1,P].T @ depot[1,N-chunk] replicates the depot row to all
        # partitions; the idle Scalar engine narrows PSUM f32 -> SBUF bf16
        dflat_s = const.tile([1, N], bf16)
        nc.sync.dma_start(dflat_s[:], dflat_h[:, :])
        ones_s = const.tile([1, P], bf16)
        nc.gpsimd.memset(ones_s[:], 1.0)
        dc01_s = const.tile([P, N], bf16)
        for c in range(4):
            cols = slice(c * 512, (c + 1) * 512)
            pt = psum.tile([P, 512], f32, tag="pb")
            nc.tensor.matmul(pt[:], ones_s[:], dflat_s[:, cols])
            nc.scalar.activation(dc01_s[:, cols], pt[:], Act.Copy)
        dbig_s = const.tile([P, NT], f32)
        nc.sync.dma_start(dbig_s[:], dbig_h[:, :])
        ident_s = const.tile([P, P], bf16)
        nc.sync.dma_start(ident_s[:], ident_h[:, :])
        nge_s = const.tile([P, NT], f32)
        t16r_s = const.tile([P, NT], f32)

        NCH = 8          # selection chunks per row
        CW = N // NCH    # 256 columns per chunk
        for r in range(NT):
            rows = slice(r * P, (r + 1) * P)
            d_t = inp.tile([P, N], f32, tag="d")
            nc.sync.dma_start(d_t[:], d_h[rows, :])
            twc_t = inp.tile([P, N], i32, tag="twc")
            nc.sync.dma_start(twc_t[:], twc_h[rows, :])

            # x = twc - d: eligible j have x in (0,1], blocked j have x <= 0,
            # so the top-16 of x = the 16 nearest eligible neighbors
            x = work.tile([P, N], f32, tag="x")
            nc.vector.tensor_tensor(x[:], twc_t[:], d_t[:], Alu.subtract)
            # exclude diagonal from selection: x_diag -= 2.5
            xblk = x[:, rows]
            nc.vector.scalar_tensor_tensor(
                xblk, ident_s[:], -2.5, xblk, Alu.mult, Alu.add
            )
            # per-chunk top-8 -> 64 candidates.  The true top-16 is contained
            # in the candidates unless one 256-chunk holds >8 of it; that rare
            # case makes the computed threshold strictly smaller, so the row
            # count comes out > 16 and the host repairs the row exactly.
            cand = small.tile([P, NCH * 8], f32, tag="cand")
            for c in range(NCH):
                nc.vector.max(cand[:, c * 8 : (c + 1) * 8],
                              x[:, c * CW : (c + 1) * CW])
            # global top-8 (always exact: a chunk top-8 covers its share)
            v8a = small.tile([P, 8], f32, tag="v8a")
            nc.vector.max(v8a[:], cand[:])
            # remove exactly those 8 from the candidates, then next-8
            cand2 = small.tile([P, NCH * 8], f32, tag="cand2")
            nc.vector.match_replace(cand2[:], v8a[:], cand[:], -1e30)
            v8b = small.tile([P, 8], f32, tag="v8b")
            nc.vector.max(v8b[:], cand2[:])
            # knn = (x >= 16th largest), nge[:, r] = per-row count
            # (diag still excluded: x_diag <= -0.5 < t16, so the count is a
            #  pure top-16 count -- 16 unless a float tie at a boundary)
            # t16' = 16th largest - 1e30*depot_row: depot rows compare all-true
            # (whole row is 1 in the reference), and the host ignores their
            # count when flagging tie rows.
            # raw 16th-largest shipped out: t16r <= 0 means the row had fewer
            # than 16 eligible neighbors (never on this data; host repairs)
            nc.vector.tensor_copy(t16r_s[:, r : r + 1], v8b[:, 7:8])
            t16 = small.tile([P, 1], f32, tag="t16")
            nc.vector.tensor_tensor(
                t16[:], v8b[:, 7:8], dbig_s[:, r : r + 1], Alu.subtract
            )
            knn = work.tile([P, N], bf16, tag="knn")
            nc.vector.tensor_scalar(
                knn[:], x[:], t16[:], None, Alu.is_ge, Alu.add,
                accum_out=nge_s[:, r : r + 1],
            )
            # ob = knn OR depot_col -- all-bf16 tensor_tensor gets the packed
            # 2x DVE mode; the idle Scalar engine widens bf16 -> f32
            ob = work.tile([P, N], bf16, tag="ob")
            nc.vector.tensor_tensor(ob[:], knn[:], dc01_s[:], Alu.logical_or)
            # force the diagonal on
            oblk = ob[:, rows]
            nc.vector.tensor_tensor(oblk, oblk, ident_s[:], Alu.logical_or)
            out_t = outp.tile([P, N], f32, tag="out")
            for h in range(2):
                cols = slice(h * (N // 2), (h + 1) * (N // 2))
                nc.scalar.activation(out_t[:, cols], ob[:, cols], Act.Copy)
                nc.scalar.dma_start(mask_h[rows, cols], out_t[:, cols])

        nc.scalar.dma_start(nge_h[:, :], nge_s[:])
        nc.scalar.dma_start(t16r_h[:, :], t16r_s[:])

    nc.compile()
    _program_cache["nc"] = nc
    return nc


def _repair_row(d_row, twc_row, depot_b, max_dist_b, i):
    """Exact float32 re-computation of reference row i (handles ties)."""
    n = d_row.shape[0]
    m = (twc_row == 0).astype(np.float32)
    m[i] = np.float32(1.0)
    big = (m * np.float32(max_dist_b)) * np.float32(10.0)
    dist = d_row * (np.float32(1.0) - m) + big
    idx = np.argsort(dist, kind="stable")[:K]
    knn = np.zeros(n, np.float32)
    knn[idx] = 1.0
    knn *= (twc_row == 1)
    dep = (depot_b + depot_b[i]) > 0
    out = ((knn > 0) | dep | (np.arange(n) == i)).astype(np.float32)
    return out


def make_in_maps(distance_matrix, time_window_compatibility, depot):
    bf = mybir.dt.np(bf16)
    ident = np.eye(P, dtype=bf)
    in_maps = []
    for b in range(B):
        dep_f = depot[b].astype(np.float32)
        in_maps.append({
            "d": distance_matrix[b],
            "twc": time_window_compatibility[b],
            "dflat": np.ascontiguousarray(dep_f.astype(bf).reshape(1, N)),
            "dbig": np.ascontiguousarray(
                (dep_f * np.float32(1e30)).reshape(NT, P).T),
            "ident": ident,
        })
    return in_maps


def kernel(distance_matrix, max_dist, time_window_compatibility, depot,
           num_neighbors_encoder):
    distance_matrix = np.asarray(distance_matrix)
    time_window_compatibility = np.asarray(time_window_compatibility)
    depot = np.asarray(depot)
    max_dist = np.asarray(max_dist).reshape(B)
    assert int(np.asarray(num_neighbors_encoder)) == K
    assert distance_matrix.shape == (B, N, N)

    nc = build_program()
    in_maps = make_in_maps(distance_matrix, time_window_compatibility, depot)
    res = run_bass_kernel_spmd(nc, in_maps, list(range(B)), trace=False)
    _program_cache["last_results"] = res

    out = np.stack([res.results[b]["mask"] for b in range(B)])
    nge = np.stack([res.results[b]["nge"] for b in range(B)])  # [B, P, NT]
    t16r = np.stack([res.results[b]["t16r"] for b in range(B)])

    # exact repair of rows with a float tie at a selection boundary, or with
    # fewer than 16 eligible neighbors (t16r <= 0).  Depot rows are all-ones
    # by construction (they report count 2048) and never need repair.
    flag = ((nge != np.float32(K)) | (t16r <= 0)) & (
        depot.reshape(B, NT, P).transpose(0, 2, 1) == 0)
    for b, p, r in zip(*np.nonzero(flag)):
        i = int(r) * P + int(p)
        out[b, i] = _repair_row(
            distance_matrix[b, i], time_window_compatibility[b, i],
            depot[b], max_dist[b], i,
        )
    return out
